# revision 70
# baseline (speedup 1.0000x reference)
"""BesselKAN layer kernel for Trainium2 (8 NeuronCores, data-parallel batch).

reference math:
    t = tanh(x)                                   # [B, I]
    b0 = 1; b1 = 1+t; b2 = 1+3t+3t^2; b3 = 1+6t+15t^2+15t^3
    y[b,o] = sum_{i,d} b_d[b,i] * W[i,o,d]        # W = bessel_coeffs [I, O, 4]

Monomial rewrite (exact algebra):
    y = bias + t @ C1 + (3 t^2) @ C2 + (15 t^3) @ C3
    C1 = W1 + 3 W2 + 6 W3 ; C2 = W2 + 5 W3 ; C3 = W3
    bias_o = colsum(W0 + W1 + W2 + W3)

Device strategy (per core, 1024 batch rows):
  - All contraction matmuls run as fp8e4m3 DoubleRow (2x PE rate, 0.5
    cycles/column).  The t^2/t^3 terms carry most of the signal, so
    their operands are split hi/lo: u = q(u) + r, C = q(C) + r(C), and
    three DR passes per term (q@q, r@q, q@r) recover ~bf16 accuracy at
    1.5x the cost of one fp8 pass instead of 2x (bf16).  The t term is
    small enough for a single plain fp8 pass.
  - Weight-side planes are host-prepared (layout permute + monomial fold
    + 2^13 pre-scale + saturating fp8 casts): c1q, c2q/c2r, c3q/c3r,
    ssq/ssr.  The 8192x pre-scale (exact power of two) keeps the fp8
    residual planes out of the subnormal range; the final yo op
    descales by 1/8192.
  - x is host-transposed (and bf16-cast) so tanh lands directly in
    [i_part, b] layout - no PE transposes anywhere.  u-side pipeline per
    ki: tanh (ACT), 3t^2 = Square(sqrt3 * t) (ACT), 15t^3 (DVE stt),
    fp8 casts (DVE/GpSimd tensor_copy), residuals u - q(u) (DVE/GpSimd
    tensor_tensor subtract; GPSIMD cannot run TensorScalarPtr or touch
    PSUM on trn2).
  - bias: colsum of ssq+ssr via DR matmuls with an all-ones fp8
    stationary; added during the PSUM drain: yo = po/8192 + bias
    (DVE scalar_tensor_tensor, out bf16) -> DMA; host upcasts to f32.
  - emission: ki-pair-major over an "A" superwave of 7 PSUM groups
    whose columns are ordered by estimated operand availability
    (sched="auto"), so PE consumption paces the ACT/DVE/GpSimd
    elementwise streams; remaining groups run group-major afterwards,
    closing staggered so yo/DMA overlap PE.
"""

import sys
from contextlib import ExitStack

import numpy as np

if "/opt/trn_rl_repo" not in sys.path:
    sys.path.insert(0, "/opt/trn_rl_repo")

import ml_dtypes

import concourse.bass as bass
import concourse.tile as tile
from concourse import bacc, mybir
from concourse._compat import with_exitstack

P = 128
N_CORES = 8
B_FULL = 8192
I_DIM = 1024
O_DIM = 1024
NDEG = 4

FP32 = mybir.dt.float32
BF16 = mybir.dt.bfloat16
FP8 = mybir.dt.float8e4

SQRT3 = float(np.sqrt(3.0))
WSCALE = 8192.0  # weight-plane pre-scale (2^13, exact)

BF16_NP = ml_dtypes.bfloat16
FP8_NP = ml_dtypes.float8_e4m3

MULT = mybir.AluOpType.mult
ADD = mybir.AluOpType.add
SUB = mybir.AluOpType.subtract

DEFAULT_CFG = dict(
    xbufs=4,
    wbufs=4,
    yobufs=4,
    pbufs=8,
    wave_a=7,
    bias_shared_pool=True,
    nsplit=1,
    nsplit_kis=2,
    resid_engine="gpsimd",  # or "vector"
    yo_engine="vector",  # vector | scalar2 (gpsimd cannot read PSUM)
    sched="auto",
    w_chunk=True,
    colsum_at=99,
    companions=0,
    splits=0,
    split_from=3,
    tailpipe=0,
    warmup=0,
    w_order=("c1q0 c2q0 c3q0 c2r0 c3r0 c1q0 c2q0 c3q0 c2r0 c3r0 "
             "ssq0 ssr0 c1q1 c2q1 c3q1 c2r1 c3r1 ssq1 ssr1"),
)


@with_exitstack
def _bessel_body(ctx: ExitStack, tc: "tile.TileContext", y_d, xt_d, wplanes_d,
                 b_loc, i_dim, o_dim, cfg=None):
    """wplanes_d: dict name -> dram AP, each [P, KI, o_dim]:
    c1q(fp8) c2q c2r c3q c3r (fp8) ssq ssr (fp8)."""
    cfg = {**DEFAULT_CFG, **(cfg or {})}
    nc = tc.nc
    KI = i_dim // P
    KP = KI // 2
    NJ = b_loc // P
    OW = min(512, o_dim)
    OH = o_dim // OW
    resid = nc.gpsimd if cfg["resid_engine"] == "gpsimd" else nc.vector

    singles = ctx.enter_context(tc.tile_pool(name="singles", bufs=1))
    xpool = ctx.enter_context(tc.tile_pool(name="xpool", bufs=cfg["xbufs"]))
    wpool = ctx.enter_context(tc.tile_pool(name="wpool", bufs=cfg["wbufs"]))
    yopool = ctx.enter_context(tc.tile_pool(name="yopool", bufs=cfg["yobufs"]))
    psum_o = ctx.enter_context(
        tc.tile_pool(name="psum_o", bufs=cfg["pbufs"], space="PSUM"))
    psum_b = psum_o if cfg["bias_shared_pool"] else ctx.enter_context(
        tc.tile_pool(name="psum_b", bufs=1, space="PSUM"))

    # constants
    ones_dr = singles.tile([P, 2, P], FP8, name="ones_dr")
    nc.vector.memset(ones_dr[:], 1.0)
    e_row = singles.tile([P, P], BF16, name="e_row")
    nc.vector.memset(e_row[:], 0.0)
    nc.vector.memset(e_row[0:1, :], 1.0)

    # u-side persistent tensors, [i_part, ki, b]
    u1b = singles.tile([P, KI, b_loc], BF16, name="u1b")
    u1q = singles.tile([P, KI, b_loc], FP8, name="u1q")
    u2b = singles.tile([P, KI, b_loc], BF16, name="u2b")
    u2q = singles.tile([P, KI, b_loc], FP8, name="u2q")
    u2r = singles.tile([P, KI, b_loc], FP8, name="u2r")
    u3b = singles.tile([P, KI, b_loc], BF16, name="u3b")
    u3q = singles.tile([P, KI, b_loc], FP8, name="u3q")
    u3r = singles.tile([P, KI, b_loc], FP8, name="u3r")

    # weight-side persistent fp8 tiles per output half
    W_NAMES = ("c1q", "c2q", "c2r", "c3q", "c3r")
    wsb = {(n, oh): singles.tile([P, KI, OW], FP8, name=f"{n}_{oh}")
           for n in W_NAMES for oh in range(OH)}
    sssb = {(n, oh): singles.tile([P, KI, OW], FP8, name=f"{n}_{oh}")
            for n in ("ssq", "ssr") for oh in range(OH)}
    bias_sb = [singles.tile([P, OW], BF16, name=f"bias_sb{oh}")
               for oh in range(OH)]
    bias_sc = [singles.tile([P, OW], BF16, name=f"bias_sc{oh}")
               for oh in range(OH)]

    def emit_u_pair(kp, nsplit=1, wtake=()):
        # Emit both kis of a contraction pair with ops grouped by matmul-pass
        # consumption priority (u1q -> u2q -> u3q -> residuals) so each
        # engine's in-order queue produces pair-complete tensors asap.
        # nsplit>1 additionally halves the b-range per op for shorter chain
        # latency at kernel startup.
        kis = [2 * kp, 2 * kp + 1][:max(1, KI - 2 * kp)]
        xts = {}
        for ki in kis:
            xts[ki] = xpool.tile([P, b_loc], BF16, tag="x_t", name=f"x_t{ki}")
        step = b_loc // nsplit
        for s in range(nsplit):
            for _ in range(wtake[s] if s < len(wtake) else 0):
                if wq:
                    emit_wdma(*wq.pop(0))
            bsl = slice(s * step, (s + 1) * step)

            def sl(ki):
                return (slice(None), ki, bsl)

            for ki in kis:
                nc.sync.dma_start(out=xts[ki][:, bsl], in_=xt_d[:, ki, bsl])
                nc.scalar.activation(out=u1b[sl(ki)], in_=xts[ki][:, bsl],
                                     func=mybir.ActivationFunctionType.Tanh)
            for ki in kis:
                nc.scalar.activation(out=u2b[sl(ki)], in_=u1b[sl(ki)],
                                     func=mybir.ActivationFunctionType.Square,
                                     scale=SQRT3)
            for ci, ki in enumerate(kis):
                eng = nc.vector if ci % 2 == 0 else nc.gpsimd
                eng.tensor_copy(out=u2q[sl(ki)], in_=u2b[sl(ki)])
            for ki in kis:
                nc.scalar.activation(out=u1q[sl(ki)], in_=xts[ki][:, bsl],
                                     func=mybir.ActivationFunctionType.Tanh)
            for ki in kis:
                nc.vector.scalar_tensor_tensor(
                    out=u3b[sl(ki)], in0=u1b[sl(ki)], scalar=5.0,
                    in1=u2b[sl(ki)], op0=MULT, op1=MULT)
                nc.vector.tensor_copy(out=u3q[sl(ki)], in_=u3b[sl(ki)])
            for ki in kis:
                nc.vector.tensor_tensor(out=u2r[sl(ki)], in0=u2b[sl(ki)],
                                        in1=u2q[sl(ki)], op=SUB)
            for ki in kis:
                nc.gpsimd.tensor_tensor(out=u3r[sl(ki)], in0=u3b[sl(ki)],
                                        in1=u3q[sl(ki)], op=SUB)

    def emit_wdma(name, oh, kis=None):
        dst = sssb[(name, oh)] if name in ("ssq", "ssr") else wsb[(name, oh)]
        kis = kis or (0, KI)
        nc.sync.dma_start(
            out=dst[:, kis[0]:kis[1], :],
            in_=wplanes_d[name][:, kis[0]:kis[1], oh * OW:(oh + 1) * OW])

    # ---- emission: u-prep interleaved with W DMAs (program order ~ priority).
    # q-planes stream first (they gate the early matmul passes), residual
    # planes next, ss planes last (bias is only needed at group close).
    # The first c3q/c2q chunks are split so the first passes' operands land
    # with the first x tiles.
    kc = min(2, KI)
    W_CHUNKED = {"c1q", "c2q", "c3q", "c2r", "c3r"}

    def worder():
        # cfg string: space-separated "<plane><oh>" tokens; chunked planes
        # expand to (0,kc) + (kc,KI) at their first/second occurrence
        seen = set()
        out = []
        for tok in cfg["w_order"].split():
            n, oh = tok[:3], int(tok[3])
            if oh >= OH:
                continue
            if n in W_CHUNKED and cfg["w_chunk"] and oh == 0:
                if (n, oh) not in seen:
                    out.append((n, oh, (0, kc)))
                    seen.add((n, oh))
                else:
                    out.append((n, oh, (kc, KI)))
            elif (n, oh) not in seen:
                out.append((n, oh, None))
                seen.add((n, oh))
        emitted = {}
        for n, oh, k in out:
            lo, hi = k if k else (0, KI)
            emitted[(n, oh)] = max(emitted.get((n, oh), 0), hi)
        need = [(n, oh) for n in
                ("c1q", "c2q", "c2r", "c3q", "c3r", "ssq", "ssr")
                for oh in range(OH)]
        for n, oh in need:
            hi = emitted.get((n, oh), 0)
            if hi < KI:
                out.append((n, oh, (hi, KI)))
        return [e for e in out if e[2] is None or e[2][0] < e[2][1]]

    wq = worder()
    for kp in range(KP):
        if kp == 0:
            emit_u_pair(kp, nsplit=cfg["nsplit"], wtake=(0, 2))
            take = 2
        else:
            emit_u_pair(kp)
            take = 2
        for _ in range(take):
            if wq:
                emit_wdma(*wq.pop(0))
    while wq:
        emit_wdma(*wq.pop(0))

    # (u fp8 tensor, weight plane name) passes per accumulation group,
    # ordered by when the operands become available (q before residual)
    PASSES = (
        ("t1qq", u1q, "c1q"), ("t2qq", u2q, "c2q"), ("t3qq", u3q, "c3q"),
        ("t2qr", u2q, "c2r"), ("t3qr", u3q, "c3r"),
        ("t2rq", u2r, "c2q"), ("t3rq", u3r, "c3q"),
    )

    def emit_colsum(oh):
        # bias: colsum of ssq+ssr via DR matmuls with all-ones stationary;
        # bias_sb holds bias/WSCALE (the yo stt adds it after descale)
        bias_ps = psum_b.tile([P, OW], FP32, tag="po",
                              name=f"bias_ps{oh}")
        n_cs = 2 * KP
        ci = 0
        for src in ("ssq", "ssr"):
            for kp in range(KP):
                nc.tensor.matmul(
                    bias_ps[:], ones_dr[:],
                    sssb[(src, oh)][:, 2 * kp:2 * kp + 2, :],
                    start=(ci == 0), stop=(ci == n_cs - 1),
                    perf_mode=mybir.MatmulPerfMode.DoubleRow)
                ci += 1
        nc.scalar.activation(out=bias_sb[oh][:], in_=bias_ps[:],
                             func=mybir.ActivationFunctionType.Copy,
                             scale=1.0 / WSCALE)
        if cfg["splits"]:
            nc.scalar.activation(out=bias_sc[oh][:], in_=bias_ps[:],
                                 func=mybir.ActivationFunctionType.Copy)

    def mm(po, u, cname, oh, kp, j, start, stop=False):
        nc.tensor.matmul(
            po[:],
            u[:, 2 * kp:2 * kp + 2, j * P:(j + 1) * P],
            wsb[(cname, oh)][:, 2 * kp:2 * kp + 2, :],
            start=start, stop=stop,
            perf_mode=mybir.MatmulPerfMode.DoubleRow)

    close_n = [0]

    def emit_close(po, oh, j, split=1):
        # yo = po/WSCALE + bias, alternating between the two late-phase-idle
        # elementwise engines so closes never serialize.  split>1 chops the
        # close into parallel column strips (tail-latency reduction for the
        # last groups).
        yo = yopool.tile([P, OW], BF16, tag="yo")
        step = OW // split
        for s in range(split):
            c = slice(s * step, (s + 1) * step)
            eng = yengs[close_n[0] % len(yengs)]
            close_n[0] += 1
            eng.scalar_tensor_tensor(out=yo[:, c], in0=po[:, c],
                                     scalar=1.0 / WSCALE,
                                     in1=bias_sb[oh][:, c],
                                     op0=MULT, op1=ADD)
            nc.sync.dma_start(
                out=y_d[j * P:(j + 1) * P,
                        oh * OW + s * step:oh * OW + (s + 1) * step],
                in_=yo[:, c])

    yengs = {"vector": (nc.vector,)}[cfg["yo_engine"]]

    # phase A (u-prep-gated): first NA groups of oh 0, ki-pair-major so PE
    # consumption paces with u production.  phase B (free-running): the
    # rest, group-major so groups close staggered and yo/DMA overlap PE.
    # PE warm-up: free dummy matmuls during the startup window keep the
    # tensor engine past its pstate ramp before the real stream begins
    if cfg["warmup"]:
        wmov = singles.tile([P, 2, OW], FP8, name="wmov")
        nc.vector.memset(wmov[:], 1.0)
        wu_ps = psum_o.tile([P, OW], FP32, tag="po", name="wu_ps")
        for wi in range(cfg["warmup"]):
            nc.tensor.matmul(wu_ps[:], ones_dr[:], wmov[:],
                             start=(wi == 0), stop=(wi == cfg["warmup"] - 1),
                             perf_mode=mybir.MatmulPerfMode.DoubleRow)

    NA = min(cfg["wave_a"], NJ)
    NC_ = min(cfg["companions"], NJ - NA)  # q-only companion groups
    pos_a = {j: psum_o.tile([P, OW], FP32, tag="po", name=f"poA{j}")
             for j in range(NA + NC_)}
    # availability-ordered (pass-group, kp) interleave: q-passes stream off
    # DVE (fast), r-passes off the residual engine (slower).  Companion
    # groups join only the q-columns (their r-columns run at phase-B start,
    # filling early PE gaps without extra PSUM pressure later).
    qs, rs = PASSES[:3], PASSES[3:]
    if cfg["sched"] == "auto":
        # sort single-pass columns by estimated operand availability
        # (pair-rate ~6.9us on DVE/Pool; offsets from per-pair queue order)
        OFFS = {"t1qq": 6.3, "t2qq": 2.9, "t3qq": 4.7, "t2qr": 2.9,
                "t3qr": 4.7, "t2rq": 6.3, "t3rq": 6.3}
        PLANE = {"t1qq": 2.0, "t2qq": 1.0, "t3qq": 3.0, "t2qr": 5.0,
                 "t3qr": 6.0, "t2rq": 1.0, "t3rq": 3.0}
        cols = []
        for pi, p in enumerate(PASSES):
            for kp in range(KP):
                est = max(4.0 + 6.9 * kp + OFFS[p[0]], 2.0 + PLANE[p[0]])
                cols.append((est, kp, pi, p))
        cols.sort(key=lambda c: (c[0], c[1]))
        sched = [((p,), kp) for _, kp, _, p in cols]
    else:
        sched = [(qs, int(c[1:])) if c[0] == "q" else (rs, int(c[1:]))
                 for c in cfg["sched"].split()]
        sched = [(grp, kp) for grp, kp in sched if kp < KP]
    def grp_is_q(g):
        return all(p[0].endswith("qq") or p[0].endswith("qr") for p in g) \
            and g is not rs
    n_q = sum(len(g) for g, _ in sched if grp_is_q(g))
    n_r = sum(len(g) for g, _ in sched if not grp_is_q(g))
    total = {j: (n_q + n_r if j < NA else n_q + len(rs) * KP)
             for j in range(NA + NC_)}
    done = {j: 0 for j in range(NA + NC_)}

    def mm_a(j, u, cname, kp):
        done[j] += 1
        mm(pos_a[j], u, cname, 0, kp, j, start=(done[j] == 1),
           stop=(done[j] == total[j]))

    # Split groups: output tiles whose kp0..KPH-1 contribution runs during
    # phase A in a rotating PSUM bank, parked to an SBUF partial (ACT copy,
    # bank freed) and merged at the final close.  Gives PE fill work while
    # the u streams pace phase A.
    all_groups = [(0, j) for j in range(NJ)]
    all_groups += [(1, j) for j in range(NJ)] if OH > 1 else []
    split_groups = ([g for g in all_groups[NA + NC_:]][:cfg["splits"]]
                    if KP >= 2 else [])
    KPH = max(1, KP // 2)
    partials = {}

    def emit_split_early(oh, j):
        po = psum_o.tile([P, OW], FP32, tag="po", name=f"poS{oh}_{j}")
        n = 0
        for kp in range(KPH):
            for pi, (_, u, cname) in enumerate(PASSES):
                n += 1
                mm(po, u, cname, oh, kp, j, start=(n == 1),
                   stop=(n == KPH * len(PASSES)))
        part = singles.tile([P, OW], BF16, name=f"part{oh}_{j}")
        nc.scalar.activation(out=part[:], in_=po[:],
                             func=mybir.ActivationFunctionType.Copy,
                             scale=1.0 / WSCALE)
        partials[(oh, j)] = part

    def emit_split_final(oh, j):
        po = psum_o.tile([P, OW], FP32, tag="po", name=f"poF{oh}_{j}")
        n = 0
        for kp in range(KPH, KP):
            for pi, (_, u, cname) in enumerate(PASSES):
                n += 1
                mm(po, u, cname, oh, kp, j, start=(n == 1))
        nc.tensor.matmul(po[:], e_row[:], bias_sc[oh][:], start=False,
                         stop=True)
        yo = yopool.tile([P, OW], BF16, tag="yo")
        nc.vector.scalar_tensor_tensor(
            out=yo[:], in0=po[:], scalar=1.0 / WSCALE,
            in1=partials[(oh, j)][:], op0=MULT, op1=ADD)
        nc.sync.dma_start(
            out=y_d[j * P:(j + 1) * P, oh * OW:(oh + 1) * OW], in_=yo[:])

    # tail-pipeline: the first non-A group's columns that don't depend on
    # the last-arriving residuals run right before phase A's final columns,
    # filling the stream-tail PE gap (uses the one spare PSUM buffer)
    b0 = all_groups[NA + NC_] if (cfg["tailpipe"] and KP >= 2
                                  and len(all_groups) > NA + NC_
                                  and not split_groups) else None
    tp_si = max(0, len(sched) - cfg["tailpipe"]) if b0 else None
    po_b0 = None
    nb0 = 0

    splits_iter = list(split_groups)
    for si, (grp, kp) in enumerate(sched):
        if b0 is not None and si == tp_si:
            po_b0 = psum_o.tile([P, OW], FP32, tag="po",
                                name=f"poTP{b0[0]}_{b0[1]}")
            for grp2, kp2 in sched[:tp_si]:
                for _, u2, cn2 in grp2:
                    nb0 += 1
                    mm(po_b0, u2, cn2, b0[0], kp2, b0[1], start=(nb0 == 1))
        is_q = grp_is_q(grp)
        for _, u, cname in grp:
            for j in range(NA + (NC_ if is_q else 0)):
                mm_a(j, u, cname, kp)
        if si == cfg["colsum_at"]:
            emit_colsum(0)
        if si >= cfg["split_from"] and splits_iter:
            oh, j = splits_iter.pop(0)
            emit_split_early(oh, j)
    if cfg["colsum_at"] >= len(sched):
        emit_colsum(0)
    while splits_iter:
        emit_split_early(*splits_iter.pop(0))
    # companions: finish their r-columns, then close everything
    for kp in range(KP):
        for _, u, cname in rs:
            for j in range(NA, NA + NC_):
                mm_a(j, u, cname, kp)
    if b0 is not None:
        n_total = sum(len(g) for g, _ in sched)
        for grp2, kp2 in sched[tp_si:]:
            for _, u2, cn2 in grp2:
                nb0 += 1
                mm(po_b0, u2, cn2, b0[0], kp2, b0[1], start=False,
                   stop=(nb0 == n_total))
    for j in range(NA + NC_):
        emit_close(pos_a[j], 0, j)
    if b0 is not None:
        emit_close(po_b0, b0[0], b0[1])

    if OH > 1:
        emit_colsum(1)
    for oh, j in split_groups:
        emit_split_final(oh, j)
    for gi, (oh, j) in enumerate(all_groups[NA + NC_:]):
        if (oh, j) in partials or (b0 is not None and (oh, j) == b0):
            continue
        po = psum_o.tile([P, OW], FP32, tag="po", name=f"poB{oh}_{j}")
        for kp in range(KP):
            for pi, (_, u, cname) in enumerate(PASSES):
                mm(po, u, cname, oh, kp, j, start=(kp == 0 and pi == 0),
                   stop=(kp == KP - 1 and pi == len(PASSES) - 1))
        emit_close(po, oh, j)


W_PLANE_NAMES = ("c1q", "c2q", "c2r", "c3q", "c3r", "ssq", "ssr")


def build_nc(b_loc=B_FULL // N_CORES, i_dim=I_DIM, o_dim=O_DIM,
             n_cores=N_CORES, cfg=None):
    nc = bacc.Bacc("TRN2", target_bir_lowering=False, debug=False,
                   num_devices=n_cores)
    KI = i_dim // P
    xt_d = nc.dram_tensor("xt", [P, KI, b_loc], BF16,
                          kind="ExternalInput").ap()
    wplanes_d = {
        name: nc.dram_tensor(name, [P, KI, o_dim], FP8,
                             kind="ExternalInput").ap()
        for name in W_PLANE_NAMES
    }
    y_d = nc.dram_tensor("y", [b_loc, o_dim], BF16, kind="ExternalOutput").ap()
    with tile.TileContext(nc) as tc:
        _bessel_body(tc, y_d, xt_d, wplanes_d, b_loc, i_dim, o_dim, cfg=cfg)
    nc.compile()
    return nc


def prep_inputs(x, w, n_cores=N_CORES):
    """Host-side data prep: shard/permute x, fold + cast weight planes."""
    x = np.asarray(x, dtype=np.float32)
    w = np.asarray(w, dtype=np.float32)
    b_full, i_dim = x.shape
    o_dim = w.shape[1]
    KI = i_dim // P
    b_loc = b_full // n_cores

    # x^T permuted to [p, ki, b] (i = ki*P + p), cast bf16 (halves DMA)
    xt = np.ascontiguousarray(
        x.T.reshape(KI, P, b_full).transpose(1, 0, 2)).astype(BF16_NP)

    w64 = w.astype(np.float64)
    W0, W1, W2, W3 = (w64[..., d] for d in range(4))
    planes64 = {
        "c1": W1 + 3 * W2 + 6 * W3,
        "c2": W2 + 5 * W3,
        "c3": W3,
        "ss": W0 + W1 + W2 + W3,
    }

    def perm(a):  # [I, O] -> [p, ki, O]
        return np.ascontiguousarray(
            a.reshape(KI, P, o_dim).transpose(1, 0, 2))

    def to_fp8(a):  # saturating e4m3 cast (TRN max normal 240)
        return np.clip(a, -240.0, 240.0).astype(FP8_NP)

    out = {"c1q": perm(to_fp8(WSCALE * planes64["c1"]))}
    for name in ("c2", "c3", "ss"):
        hi64 = WSCALE * planes64[name]
        q = to_fp8(hi64)
        r = to_fp8(hi64 - q.astype(np.float64))
        out[name + "q"] = perm(q)
        out[name + "r"] = perm(r)

    in_maps = []
    for c in range(n_cores):
        m = {"xt": np.ascontiguousarray(
            xt[:, :, c * b_loc:(c + 1) * b_loc])}
        m.update(out)
        in_maps.append(m)
    return in_maps


_NC_CACHE = {}


def _get_nc():
    if "full" not in _NC_CACHE:
        _NC_CACHE["full"] = build_nc()
    return _NC_CACHE["full"]


def run_spmd(x, bessel_coeffs, trace=False, **kwargs):
    from concourse.bass_utils import run_bass_kernel_spmd

    nc = _get_nc()
    in_maps = prep_inputs(x, bessel_coeffs)
    res = run_bass_kernel_spmd(nc, in_maps, core_ids=list(range(N_CORES)),
                               trace=trace, **kwargs)
    y = np.concatenate(
        [np.asarray(r["y"]).astype(np.float32) for r in res.results], axis=0)
    return y, res


def kernel(x, bessel_coeffs):
    y, _ = run_spmd(x, bessel_coeffs)
    return y.astype(np.float32)


def _ref_np(x, w):
    t = np.tanh(np.asarray(x, dtype=np.float64))
    w = np.asarray(w, dtype=np.float64)
    basis = [np.ones_like(t), t + 1.0]
    for i in range(2, NDEG):
        basis.append((2 * i - 1) * t * basis[i - 1] + basis[i - 2])
    bz = np.stack(basis, axis=-1)
    return np.einsum("bid,iod->bo", bz, w)


def _selftest_sim(b_loc=256, i_dim=256, o_dim=1024):
    """CoreSim check on a small config exercising all loop paths."""
    from concourse.bass_interp import CoreSim

    nc = build_nc(b_loc=b_loc, i_dim=i_dim, o_dim=o_dim, n_cores=1)
    rng = np.random.default_rng(0)
    x = rng.standard_normal((b_loc, i_dim)).astype(np.float32)
    w = (rng.standard_normal((i_dim, o_dim, NDEG)) / (i_dim * NDEG)).astype(
        np.float32)
    in_maps = prep_inputs(x, w, n_cores=1)
    sim = CoreSim(nc)
    for name, arr in in_maps[0].items():
        sim.tensor(name)[:] = arr
    sim.simulate()
    y = np.array(sim.tensor("y")).astype(np.float64)
    ref = _ref_np(x, w)
    scale = np.abs(ref).max()
    err = np.abs(y - ref).max() / scale
    print(f"sim scale={scale:.4g} max_abs_rel_err={err:.4g}")
    assert err < 2e-2, err
    print("SIM OK")


def _timesim(cfg=None):
    from concourse.timeline_sim import TimelineSim

    nc = build_nc(cfg=cfg)
    t = TimelineSim(nc).simulate()
    print(f"TimelineSim: {t:.0f} ns")
    return t


if __name__ == "__main__":
    if "--sim" in sys.argv:
        _selftest_sim()
    if "--timesim" in sys.argv:
        _timesim()
